# revision 18
# baseline (speedup 1.0000x reference)
"""Trainium2 Bass kernel for nn_BypassModel (segment max/mean pool + BN towers).

Contract: kernel(**inputs) takes the FULL inputs (as reference.setup_inputs),
returns the full (emotion, act, intent) tuple. Internally shards batch across
8 NeuronCores, runs one SPMD Bass program, gathers results.
"""
import sys
import numpy as np

for _p in ("/opt/trn_rl_repo", "/root/.axon_site/_ro/trn_rl_repo"):
    if _p not in sys.path:
        sys.path.append(_p)

import concourse.bass as bass  # noqa: E402
import concourse.bacc as bacc  # noqa: E402
import concourse.mybir as mybir  # noqa: E402
import concourse.tile as tile  # noqa: E402
from concourse import library_config  # noqa: E402
from concourse.bass_utils import run_bass_kernel_spmd  # noqa: E402

B, S, D, T = 64, 2048, 1024, 16
NCORE = 8
BPC = B // NCORE          # batch elems per core
SEG = BPC * T             # 128 segments per core
NS = B * T                # 1024 total segments
STEPS = 127               # max segment length
P = 128
EPS = 1e-5
HEADS = (("emo", 7), ("act", 5), ("int", 102))
f32 = mybir.dt.float32
i16 = mybir.dt.int16
AF = mybir.ActivationFunctionType

_compiled = None
_last_in_maps = None


def _build():
    nc = bacc.Bacc("TRN2", target_bir_lowering=False, debug=False,
                   num_devices=NCORE)

    hs = nc.dram_tensor("hs", [BPC * S, D], f32, kind="ExternalInput")
    idx = nc.dram_tensor("idx", [P, STEPS * 8], i16, kind="ExternalInput")
    diag = nc.dram_tensor("diag", [STEPS, P, P], f32, kind="ExternalInput")
    ident = nc.dram_tensor("ident", [P, P], f32, kind="ExternalInput")
    poolerW = nc.dram_tensor("poolerW", [P, 16, D], f32, kind="ExternalInput")
    poolerB = nc.dram_tensor("poolerB", [P, 8], f32, kind="ExternalInput")
    poolW = nc.dram_tensor("poolW", [P, 8, D], f32, kind="ExternalInput")
    poolG = nc.dram_tensor("poolG", [P, 8], f32, kind="ExternalInput")
    poolBeta = nc.dram_tensor("poolBeta", [P, 8], f32, kind="ExternalInput")
    twW, twG, twBeta, twWf, twBf = {}, {}, {}, {}, {}
    for t, (name, nout) in enumerate(HEADS):
        twW[t] = nc.dram_tensor(f"w_{name}", [4, P, 8, D], f32, kind="ExternalInput")
        twG[t] = nc.dram_tensor(f"g_{name}", [P, 4 * 8], f32, kind="ExternalInput")
        twBeta[t] = nc.dram_tensor(f"beta_{name}", [P, 4 * 8], f32, kind="ExternalInput")
        twWf[t] = nc.dram_tensor(f"wf_{name}", [P, 8, nout], f32, kind="ExternalInput")
        twBf[t] = nc.dram_tensor(f"bf_{name}", [1, nout], f32, kind="ExternalInput")
    ones_row = nc.dram_tensor("ones_row", [1, P], f32, kind="ExternalInput")
    outs = {t: nc.dram_tensor(f"o_{name}", [P, nout], f32, kind="ExternalOutput")
            for t, (name, nout) in enumerate(HEADS)}

    with tile.TileContext(nc) as tc:
        with (
            tc.tile_pool(name="const", bufs=1) as cpool,
            tc.tile_pool(name="dstream", bufs=8) as dpool,
            tc.tile_pool(name="x", bufs=3) as xpool,
            tc.tile_pool(name="wbuf", bufs=2) as wpool,
            tc.tile_pool(name="act", bufs=7) as apool,
            tc.tile_pool(name="small", bufs=12) as spool,
            tc.tile_pool(name="psA", bufs=1, space="PSUM") as psA,
            tc.tile_pool(name="psT", bufs=2, space="PSUM") as psT,
            tc.tile_pool(name="psY", bufs=2, space="PSUM") as psY,
            tc.tile_pool(name="dram", bufs=10, space="DRAM") as drampool,
        ):
            nc.gpsimd.load_library(library_config.mlp)

            # ---- constants in ----
            idx_sb = cpool.tile([P, STEPS * 8], i16)
            nc.sync.dma_start(idx_sb[:], idx[:])
            ident_sb = cpool.tile([P, P], f32)
            nc.sync.dma_start(ident_sb[:], ident[:])
            poolerB_sb = cpool.tile([P, 8], f32)
            nc.sync.dma_start(poolerB_sb[:], poolerB[:])
            poolG_sb = cpool.tile([P, 8], f32)
            nc.sync.dma_start(poolG_sb[:], poolG[:])
            poolBeta_sb = cpool.tile([P, 8], f32)
            nc.sync.dma_start(poolBeta_sb[:], poolBeta[:])
            ones_sb = cpool.tile([1, P], f32)
            nc.sync.dma_start(ones_sb[:], ones_row[:])
            eps_sb = cpool.tile([P, 1], f32)
            nc.vector.memset(eps_sb[:], EPS)
            g_sb, beta_sb, wf_sb, bf_sb = {}, {}, {}, {}
            for t, (name, nout) in enumerate(HEADS):
                g_sb[t] = cpool.tile([P, 32], f32, tag=f"g{t}", name=f"g_sb{t}")
                nc.sync.dma_start(g_sb[t][:], twG[t][:])
                beta_sb[t] = cpool.tile([P, 32], f32, tag=f"be{t}", name=f"beta_sb{t}")
                nc.sync.dma_start(beta_sb[t][:], twBeta[t][:])
                wf_sb[t] = cpool.tile([P, 8, nout], f32, tag=f"wf{t}", name=f"wf_sb{t}")
                nc.sync.dma_start(wf_sb[t][:], twWf[t][:])
                bf_sb[t] = cpool.tile([1, nout], f32, tag=f"bf{t}", name=f"bf_sb{t}")
                nc.sync.dma_start(bf_sb[t][:], twBf[t][:])

            # ---- phase A: lockstep segment pooling ----
            acc = cpool.tile([P, D], f32, tag="acc")       # running max
            mean_ps = psA.tile([P, D], f32)                # mean accumulator
            for k in range(STEPS):
                xk = xpool.tile([P, 1, D], f32, tag="xk")
                nc.gpsimd.dma_gather(
                    xk[:], hs[:], idx_sb[:, k * 8:(k + 1) * 8], P, P, D)
                xf = xk[:, 0, :]
                dg = dpool.tile([P, P], f32, tag="dg")
                nc.gpsimd.dma_start(dg[:], diag[k, :, :])
                if k == 0:
                    nc.vector.tensor_copy(acc[:], xf)
                else:
                    nc.vector.tensor_max(acc[:], acc[:], xf)
                for n in range(2):
                    nc.tensor.matmul(
                        mean_ps[:, n * 512:(n + 1) * 512], dg[:],
                        xf[:, n * 512:(n + 1) * 512],
                        start=(k == 0), stop=(k == STEPS - 1))

            # ---- phase B: build pooledT [feat, seg] ----
            mean_sb = cpool.tile([P, D], f32, tag="meansb")
            nc.vector.tensor_copy(mean_sb[:], mean_ps[:])
            pooledT = cpool.tile([P, 16, P], f32, tag="pooledT")
            for c in range(8):
                tp = psT.tile([P, P], f32, tag="tp")
                nc.tensor.transpose(tp[:], acc[:, c * P:(c + 1) * P], ident_sb[:])
                nc.scalar.copy(pooledT[:, c, :], tp[:])
                tp2 = psT.tile([P, P], f32, tag="tp")
                nc.tensor.transpose(tp2[:], mean_sb[:, c * P:(c + 1) * P], ident_sb[:])
                nc.scalar.copy(pooledT[:, 8 + c, :], tp2[:])

            # ---- phase C: pooler layer (tanh, no BN) ----
            pwa = wpool.tile([P, 8, D], f32, tag="w")
            nc.sync.dma_start(pwa[:], poolerW[:, 0:8, :])
            pwb = wpool.tile([P, 8, D], f32, tag="w")
            nc.sync.dma_start(pwb[:], poolerW[:, 8:16, :])
            y_ps = psY.tile([P, 8, P], f32, tag="yps")
            for mc in range(8):
                for kt in range(16):
                    w = pwa if kt < 8 else pwb
                    nc.tensor.matmul(
                        y_ps[:, mc, :], w[:, kt % 8, mc * P:(mc + 1) * P],
                        pooledT[:, kt, :], start=(kt == 0), stop=(kt == 15))
            xT = apool.tile([P, 8, P], f32, tag="xT")
            for mc in range(8):
                nc.scalar.activation(xT[:, mc, :], y_ps[:, mc, :], AF.Tanh,
                                     bias=poolerB_sb[:, mc:mc + 1])

            def bn_block(stats_list, y_list, g_b_cols, xout_list):
                """stats_list: list of (stats_sb, base). One AR for all."""
                stats_sb = stats_list
                ncols = len(y_list) * 16
                bounce_in = drampool.tile([P, ncols], f32, tag="bin")
                nc.gpsimd.dma_start(bounce_in[:], stats_sb[:, 0:ncols])
                bounce_out = drampool.tile([P, ncols], f32, tag="bout")
                nc.gpsimd.collective_compute(
                    "AllReduce", mybir.AluOpType.add,
                    replica_groups=[list(range(NCORE))],
                    ins=[bounce_in[:].opt()], outs=[bounce_out[:].opt()])
                sret = spool.tile([P, ncols], f32, tag="sret")
                nc.gpsimd.dma_start(sret[:], bounce_out[:])
                for i, y in enumerate(y_list):
                    gcol, bcol = g_b_cols[i]
                    mean = spool.tile([P, 8], f32, tag="mean")
                    nc.scalar.mul(mean[:], sret[:, i * 16:i * 16 + 8], 1.0 / NS)
                    ex2 = spool.tile([P, 8], f32, tag="ex2")
                    nc.scalar.mul(ex2[:], sret[:, i * 16 + 8:i * 16 + 16], 1.0 / NS)
                    msq = spool.tile([P, 8], f32, tag="msq")
                    nc.scalar.activation(msq[:], mean[:], AF.Square)
                    var = spool.tile([P, 8], f32, tag="var")
                    nc.vector.tensor_sub(var[:], ex2[:], msq[:])
                    varp = spool.tile([P, 8], f32, tag="varp")
                    nc.scalar.activation(varp[:], var[:], AF.Identity,
                                         bias=eps_sb[:, 0:1])
                    std = spool.tile([P, 8], f32, tag="std")
                    nc.scalar.activation(std[:], var[:], AF.Sqrt,
                                         bias=eps_sb[:, 0:1])
                    r0 = spool.tile([P, 8], f32, tag="r0")
                    nc.vector.reciprocal(r0[:], std[:])
                    # one Newton step for inverse sqrt (HW sqrt is low-precision):
                    # rstd = r0 * (1.5 - 0.5 * varp * r0^2)
                    t1 = spool.tile([P, 8], f32, tag="t1")
                    nc.vector.tensor_mul(t1[:], varp[:], r0[:])
                    nc.vector.tensor_mul(t1[:], t1[:], r0[:])
                    nc.vector.tensor_scalar(
                        t1[:], t1[:], -0.5, 1.5,
                        op0=mybir.AluOpType.mult, op1=mybir.AluOpType.add)
                    rstd = spool.tile([P, 8], f32, tag="rstd")
                    nc.vector.tensor_mul(rstd[:], r0[:], t1[:])
                    scl = spool.tile([P, 8], f32, tag="scl")
                    nc.vector.tensor_mul(scl[:], rstd[:], gcol)
                    mb = spool.tile([P, 8], f32, tag="mb")
                    nc.vector.tensor_mul(mb[:], mean[:], scl[:])
                    nbias = spool.tile([P, 8], f32, tag="nbias")
                    nc.vector.tensor_sub(nbias[:], bcol, mb[:])
                    for mc in range(8):
                        nc.scalar.activation(
                            xout_list[i][:, mc, :], y[:, mc, :], AF.Tanh,
                            bias=nbias[:, mc:mc + 1], scale=scl[:, mc:mc + 1])

            def layer_matmuls(xin, w_sb, y):
                for mc in range(8):
                    for kt in range(8):
                        nc.tensor.matmul(
                            y[:, mc, :], w_sb[:, kt, mc * P:(mc + 1) * P],
                            xin[:, kt, :], start=(kt == 0), stop=(kt == 7))

            def layer_stats(y, stats_sb, base, ysb):
                """Compute sum/sumsq stats and evacuate y PSUM->SBUF (ysb).
                Evacuation releases the PSUM slot without waiting on the
                collective, breaking the slot-dependency cycle."""
                nc.vector.tensor_reduce(
                    stats_sb[:, base:base + 8], y[:, :, :],
                    axis=mybir.AxisListType.X, op=mybir.AluOpType.add)
                for mc in range(8):
                    sqd = spool.tile([P, P], f32, tag="sqd")
                    nc.scalar.activation(
                        sqd[:], y[:, mc, :], AF.Square,
                        accum_out=stats_sb[:, base + 8 + mc:base + 9 + mc])
                nc.vector.tensor_copy(ysb[:], y[:])

            # ---- phase D: pool Linear_Block (BN over full batch) ----
            pw = wpool.tile([P, 8, D], f32, tag="w")
            nc.sync.dma_start(pw[:], poolW[:])
            y_pool = psY.tile([P, 8, P], f32, tag="yps")
            layer_matmuls(xT, pw, y_pool)
            st0 = spool.tile([P, 16], f32, tag="st")
            ysb0 = apool.tile([P, 8, P], f32, tag="ysb", bufs=4)
            layer_stats(y_pool, st0, 0, ysb0)
            x2T = apool.tile([P, 8, P], f32, tag="xT")
            bn_block(st0, [ysb0], [(poolG_sb[:, 0:8], poolBeta_sb[:, 0:8])], [x2T])

            # ---- phase E: towers ----
            cur = {t: x2T for t in range(3)}
            for lay in range(4):
                st = spool.tile([P, 48], f32, tag="st")
                ys = []
                for t in range(3):
                    w = wpool.tile([P, 8, D], f32, tag="w")
                    nc.sync.dma_start(w[:], twW[t][lay, :, :, :])
                    y = psY.tile([P, 8, P], f32, tag="yps")
                    layer_matmuls(cur[t], w, y)
                    ysb = apool.tile([P, 8, P], f32, tag="ysb", bufs=4,
                                     name=f"ysb{lay}_{t}")
                    layer_stats(y, st, t * 16, ysb)
                    ys.append(ysb)
                nxt = [apool.tile([P, 8, P], f32, tag="xT", name=f"nxt{lay}_{i}") for i in range(3)]
                bn_block(st, ys,
                         [(g_sb[t][:, lay * 8:lay * 8 + 8],
                           beta_sb[t][:, lay * 8:lay * 8 + 8]) for t in range(3)],
                         nxt)
                for t in range(3):
                    cur[t] = nxt[t]

            # ---- phase F: heads ----
            for t, (name, nout) in enumerate(HEADS):
                h_ps = psY.tile([P, 8, P], f32, tag="yps")
                for kt in range(8):
                    nc.tensor.matmul(h_ps[:, 0, 0:nout], cur[t][:, kt, :],
                                     wf_sb[t][:, kt, :],
                                     start=(kt == 0), stop=False)
                nc.tensor.matmul(h_ps[:, 0, 0:nout], ones_sb[:], bf_sb[t][:],
                                 start=False, stop=True)
                h_sb = spool.tile([P, P], f32, tag="hsb")
                nc.vector.tensor_copy(h_sb[:, 0:nout], h_ps[:, 0, 0:nout])
                nc.sync.dma_start(outs[t][:], h_sb[:, 0:nout])

    nc.compile()
    return nc


def _prep_core(c, hidden, starts, lens):
    """Build per-core host arrays: idx layout, diag, permutation."""
    bs = slice(c * BPC, (c + 1) * BPC)
    hs_flat = np.ascontiguousarray(hidden[bs]).reshape(BPC * S, D)
    # per-core segment list (local batch b', turn j)
    st = starts[bs].reshape(-1)              # [128] token start per segment
    ln = lens[bs].reshape(-1)                # [128]
    rowbase = (np.arange(BPC * T) // T) * S  # flat row base per segment
    flat_start = rowbase + st
    order = np.argsort(-ln, kind="stable")   # sorted desc by length
    fs, ls = flat_start[order], ln[order]

    idxv = np.empty((STEPS, P), np.int64)
    for k in range(STEPS):
        idxv[k] = fs + np.minimum(k, ls - 1)
    # idx_layout[p, k*8+s] = idxv[k, s*16 + p%16]  (16-partition wrap,
    # replicated to all 128 partitions for the 8 Q7 cores)
    il = np.empty((P, STEPS, 8), np.int16)
    for s in range(8):
        block = idxv[:, s * 16:(s + 1) * 16].T.astype(np.int16)  # [16, STEPS]
        il[:, :, s] = np.tile(block, (8, 1))
    idx_layout = il.reshape(P, STEPS * 8)
    diag = np.zeros((STEPS, P, P), np.float32)
    w = 1.0 / ls.astype(np.float32)
    rng = np.arange(P)
    for k in range(STEPS):
        vals = np.where(k < ls, w, 0.0).astype(np.float32)
        diag[k, rng, rng] = vals
    return hs_flat, idx_layout, diag, order


def kernel(**inputs):
    global _compiled, _last_in_maps
    hidden = np.asarray(inputs["hidden_states"], np.float32)
    turns = np.asarray(inputs["turns"], np.int64)
    parts = np.asarray(inputs["parts"], np.int64)
    assert (turns == T).all(), "kernel assumes turns == T"
    ends = 1 + np.cumsum(parts, axis=1)
    starts = ends - parts                     # [B, T]
    lens = parts                              # [B, T]

    # rearranged weights (host-side, contiguous for fast DMA)
    poolerW = np.ascontiguousarray(
        np.asarray(inputs["pooler_W"], np.float32).reshape(16, P, D).transpose(1, 0, 2))
    poolerB = np.ascontiguousarray(
        np.asarray(inputs["pooler_b"], np.float32).reshape(8, P).T)
    poolW = np.ascontiguousarray(
        np.asarray(inputs["pool_W"], np.float32).reshape(8, P, D).transpose(1, 0, 2))
    poolG = np.ascontiguousarray(
        np.asarray(inputs["pool_g"], np.float32).reshape(8, P).T)
    poolBeta = np.ascontiguousarray(
        np.asarray(inputs["pool_beta"], np.float32).reshape(8, P).T)
    ident = np.eye(P, dtype=np.float32)
    ones_row = np.ones((1, P), np.float32)

    base = dict(ident=ident, poolerW=poolerW, poolerB=poolerB, poolW=poolW,
                poolG=poolG, poolBeta=poolBeta, ones_row=ones_row)
    for name, nout in HEADS:
        Ws = np.asarray(inputs[name + "_Ws"], np.float32)
        base["w_" + name] = np.ascontiguousarray(
            Ws.reshape(4, 8, P, D).transpose(0, 2, 1, 3))
        gs = np.asarray(inputs[name + "_gs"], np.float32)
        base["g_" + name] = np.ascontiguousarray(
            gs.reshape(4, 8, P).transpose(2, 0, 1).reshape(P, 32))
        betas = np.asarray(inputs[name + "_betas"], np.float32)
        base["beta_" + name] = np.ascontiguousarray(
            betas.reshape(4, 8, P).transpose(2, 0, 1).reshape(P, 32))
        Wf = np.asarray(inputs[name + "_Wf"], np.float32)
        base["wf_" + name] = np.ascontiguousarray(
            Wf.reshape(8, P, nout).transpose(1, 0, 2))
        base["bf_" + name] = np.asarray(inputs[name + "_bf"], np.float32).reshape(1, nout)

    in_maps, orders = [], []
    for c in range(NCORE):
        hs_flat, idx_layout, diag, order = _prep_core(c, hidden, starts, lens)
        m = dict(base)
        m["hs"] = hs_flat
        m["idx"] = idx_layout
        m["diag"] = diag
        in_maps.append(m)
        orders.append(order)

    _last_in_maps = in_maps
    if _compiled is None:
        _compiled = _build()
    res = run_bass_kernel_spmd(_compiled, in_maps, core_ids=list(range(NCORE)))

    final = []
    for t, (name, nout) in enumerate(HEADS):
        out = np.empty((NS, nout), np.float32)
        for c in range(NCORE):
            rows = c * SEG + orders[c]        # global segment row per partition
            out[rows] = res.results[c][f"o_{name}"]
        final.append(out)
    return tuple(final)
